# revision 41
# baseline (speedup 1.0000x reference)
"""GCN layer kernel for 8 Trainium2 NeuronCores.

Computes out = segment_sum(edge_weight * (x @ W + b)[src], dst) for a fixed
problem size: 100000 nodes, 1.6M edges, 512 -> 32 features.

Strategy (v5)
-------------
Phase 1 (per core): core c owns nodes [12500c, 12500(c+1)). The host supplies
x pre-transposed in bf16 (xT [512, SHARD]); the device computes
h = x @ W + b with xT k-chunks as the stationary operand (no on-device
transposes), writing h rows as bf16 in 256B rows (cols 32:128 garbage, never
read) into h_local.

The shard rows split into 4 BANDS (3200/3200/3200/2944 rows). Band k of all
8 cores forms gather table hg_k [8*n_k, 128] bf16 (<= 25600 rows: int16-safe
indices). Each band has its own AllGather that fires as soon as phase 1
finishes that band's rows, so collectives pipeline behind phase-1 compute and
the first gathers start ~60us in.

Phase 2 (per core): edges are routed to the core owning their dst. Per
(core, band), dst nodes are sorted by in-degree and packed into tiles of 128
"slots"; chunk k holds at most one edge per slot. dma_gather calls batch up
to 64 chunks (8192 indices) and rotate across 4 SWDGE queues so descriptor
generation overlaps DMA execution. Gathered chunks are scaled by edge weight
on DVE (bf16), then accumulated per tile with PSUM matmuls using a fixed
bf16 identity as the STATIONARY operand (messages move: 32 cols per chunk).
Each band accumulates into its own PSUM columns; the host sums the four band
partials after undoing the per-band permutations.

kernel() is self-contained: it takes the full inputs, shards them, compiles
the Bass program once (cached), runs it on cores 0-7 and reassembles the
full [100000, 32] output.
"""
import os

import numpy as np
import ml_dtypes

import concourse.bacc as bacc
import concourse.bass as bass
import concourse.tile as tile
from concourse import bass_utils, mybir
from concourse.masks import make_identity
from concourse.tile import add_dep_helper

# ---- problem constants -------------------------------------------------
N_NODES = 100000
N_EDGES = 1600000
IN_F = 512
OUT_F = 32

N_CORES = 8
SHARD_RAW = 12500          # real nodes per core
SHARD = 12544              # padded: 98 tiles of 128
NTILES = SHARD // 128      # 98
N_BANDS = 4
BANDS = [3200, 3200, 3200, 2944]        # rows per band (25/25/25/23 tiles)
BAND_START = [0, 3200, 6400, 9600]
BAND_TILES = [25, 25, 25, 23]
HPAD = 128                 # h row padded to 128 bf16 = 256B
KTILES = IN_F // 128       # 4 k-blocks in phase 1

CHUNKS_PER_GROUP = 160     # chunk budget per tile group (bounds SBUF)
CHUNKS_PER_GATHER = 16     # 2048 indices per dma_gather call
MAX_TILES_PER_GROUP = 16   # one PSUM bank holds 16 tiles x 32 f32 per band
N_SWDGE_QUEUES = 4


def _to_bf16(a):
    return np.ascontiguousarray(np.asarray(a, np.float32).astype(
        ml_dtypes.bfloat16))


# ---- host prep ---------------------------------------------------------

def prepare(edge_index, edge_weight):
    dst = np.asarray(edge_index[0], dtype=np.int64)
    src = np.asarray(edge_index[1], dtype=np.int64)
    w = np.asarray(edge_weight, dtype=np.float32)

    core_s = src // SHARD_RAW
    r_loc = src % SHARD_RAW
    band = ((r_loc >= BAND_START[1]).astype(np.int64)
            + (r_loc >= BAND_START[2]) + (r_loc >= BAND_START[3]))
    bands_arr = np.asarray(BANDS, np.int64)
    starts_arr = np.asarray(BAND_START, np.int64)
    loc = core_s * bands_arr[band] + (r_loc - starts_arr[band])

    core = dst // SHARD_RAW
    dst_local = dst % SHARD_RAW

    deg = np.zeros((N_CORES, N_BANDS, SHARD), np.int64)
    np.add.at(deg, (core, band, dst_local), 1)

    order = np.argsort(-deg, axis=2, kind="stable").astype(np.int32)
    rank = np.empty_like(order)
    np.put_along_axis(
        rank, order, np.arange(SHARD, dtype=np.int32)[None, None, :], axis=2)

    deg_sorted = np.take_along_axis(deg, order.astype(np.int64), axis=2)
    tile_max = deg_sorted.reshape(N_CORES, N_BANDS, NTILES, 128)[:, :, :, 0]
    K = np.maximum(tile_max.max(axis=0), 1).astype(np.int64)   # [B, T]

    # tile groups balanced by total chunk count across bands
    per_tile_chunks = K.sum(axis=0)                            # [T]
    groups = []                                                # (t0, t1)
    t0 = 0
    while t0 < NTILES:
        t1 = t0
        n = 0
        while (t1 < NTILES and t1 - t0 < MAX_TILES_PER_GROUP
               and (n == 0 or n + per_tile_chunks[t1] <= CHUNKS_PER_GROUP)):
            n += int(per_tile_chunks[t1])
            t1 += 1
        groups.append((t0, t1))
        t0 = t1

    # chunk order: (group, band, tile, k)
    chunk_base = np.zeros((N_BANDS, NTILES), np.int64)
    nxt = 0
    for (t0, t1) in groups:
        for q in range(N_BANDS):
            for t in range(t0, t1):
                chunk_base[q, t] = nxt
                nxt += int(K[q, t])
    total_chunks = nxt
    total_slots = total_chunks * 128

    # per-edge slot position
    e_rank = rank[core, band, dst_local].astype(np.int64)
    e_tile = e_rank // 128
    e_slot = e_rank % 128
    key = (core * N_BANDS + band) * SHARD + dst_local
    sort_idx = np.argsort(key, kind="stable")
    key_sorted = key[sort_idx]
    first = np.ones(len(key_sorted), bool)
    first[1:] = key_sorted[1:] != key_sorted[:-1]
    run_start = np.maximum.accumulate(
        np.where(first, np.arange(len(key_sorted)), 0))
    k_sorted = np.arange(len(key_sorted)) - run_start
    e_k = np.empty(len(key), np.int64)
    e_k[sort_idx] = k_sorted

    e_pos = (chunk_base[band, e_tile] + e_k) * 128 + e_slot

    idx_flat = np.zeros((N_CORES, total_slots), np.int16)
    w_flat = np.zeros((N_CORES, total_slots), np.float32)
    idx_flat[core, e_pos] = loc.astype(np.int16)
    w_flat[core, e_pos] = w

    # gather instruction meta: per (group, band): (idx col offset, nidx, nch)
    gather_meta = []
    icol = 0
    for (t0, t1) in groups:
        per_q = []
        for q in range(N_BANDS):
            nch = int(K[q, t0:t1].sum())
            nidx = nch * 128
            per_q.append((icol, nidx, nch))
            icol += nidx // 16
        gather_meta.append(per_q)

    # idx wrapped by 16 per (group, band) block, concatenated along cols.
    # slot layout is already (group, band, tile, k, slot) contiguous.
    idx_wrapped = (idx_flat.reshape(N_CORES, total_slots // 16, 16)
                   .transpose(0, 2, 1))                        # [C, 16, S]
    idx_in = np.ascontiguousarray(
        np.tile(idx_wrapped, (1, 8, 1)))                       # [C, 128, S]

    # per-chunk weights, broadcast layout [C, 128 slots, chunks], bf16
    w_in = np.ascontiguousarray(
        _to_bf16(w_flat).reshape(N_CORES, total_chunks, 128).transpose(0, 2, 1))

    return dict(K=K, groups=groups, chunk_base=chunk_base,
                total_chunks=total_chunks, total_slots=total_slots,
                gather_meta=gather_meta, idx_in=idx_in, w_in=w_in,
                order=order)


# ---- device program ----------------------------------------------------

def build_nc(K, groups, gather_meta, total_chunks, total_slots):
    nc = bacc.Bacc("TRN2", target_bir_lowering=False, debug=False,
                   num_devices=N_CORES, num_swdge_queues=N_SWDGE_QUEUES)
    f32 = mybir.dt.float32
    bf16 = mybir.dt.bfloat16
    # host supplies xT bf16: [KTILES, 128, SHARD]
    x_in = nc.dram_tensor("xT", [KTILES, 128, SHARD], bf16,
                          kind="ExternalInput")
    wgt_in = nc.dram_tensor("wgt", [KTILES, 128, OUT_F], bf16,
                            kind="ExternalInput")
    bias_in = nc.dram_tensor("bias_r", [128, OUT_F], f32, kind="ExternalInput")
    idx_in = nc.dram_tensor("idx", [128, total_slots // 16], mybir.dt.int16,
                            kind="ExternalInput")
    w_in = nc.dram_tensor("wcol", [128, total_chunks], bf16,
                          kind="ExternalInput")
    # per-band partial outputs, partition-major: [band, p, tile, f];
    # node 128*t + p of band q lives at partial[q, p, t, :]
    partial = nc.dram_tensor("partial", [N_BANDS, 128, NTILES, OUT_F], f32,
                             kind="ExternalOutput")
    h_dbg = None
    if os.environ.get("KM_DEBUG_H"):
        h_dbg = nc.dram_tensor("h_dbg", [SHARD, OUT_F], bf16,
                               kind="ExternalOutput")

    with tile.TileContext(nc) as tc:
        with tc.tile_pool(name="dram", bufs=1, space="DRAM") as dram, \
             tc.tile_pool(name="const", bufs=1) as const:
            ident = const.tile([128, 128], bf16)
            make_identity(nc, ident[:])
            wgt = const.tile([128, KTILES, OUT_F], bf16)
            for j in range(KTILES):
                nc.sync.dma_start(wgt[:, j, :], wgt_in[j])
            bias_sb = const.tile([128, OUT_F], f32)
            nc.sync.dma_start(bias_sb[:], bias_in[:])

            h_local = dram.tile([SHARD, OUT_F], bf16)
            hgc = [dram.tile([N_CORES * BANDS[k], OUT_F], bf16,
                             addr_space="Shared", name=f"hgc{k}")
                   for k in range(N_BANDS)]
            hg = [dram.tile([N_CORES * BANDS[k], HPAD], bf16,
                            name=f"hg{k}")
                  for k in range(N_BANDS)]

            # ------ phase 1 (banded) + pipelined per-band allgather ------
            ccs = []
            with tc.tile_pool(name="p1x", bufs=2) as p1x, \
                 tc.tile_pool(name="p1psum", bufs=4, space="PSUM") as p1p, \
                 tc.tile_pool(name="p1out", bufs=2) as p1o:
                for k in range(N_BANDS):
                    bt0 = BAND_START[k] // 128
                    nbt = BAND_TILES[k]
                    xt = p1x.tile([128, KTILES, BANDS[0]], bf16, tag="xt")
                    for j in range(KTILES):
                        nc.sync.dma_start(
                            xt[:, j, 0:BANDS[k]],
                            x_in[j, :,
                                 BAND_START[k]:BAND_START[k] + BANDS[k]])
                    hb = None
                    for ti in range(nbt):
                        t = bt0 + ti
                        h_ps = p1p.tile([128, OUT_F], f32, space="PSUM",
                                        tag="h_ps")
                        for j in range(KTILES):
                            nc.tensor.matmul(
                                out=h_ps[:],
                                lhsT=xt[:, j, 128 * ti:128 * (ti + 1)],
                                rhs=wgt[:, j, :],
                                start=(j == 0), stop=(j == KTILES - 1))
                        if ti % 4 == 0:
                            hb = p1o.tile([128, 4, OUT_F], bf16, tag="hb")
                        nc.vector.tensor_tensor(
                            out=hb[:, ti % 4, :], in0=h_ps[:],
                            in1=bias_sb[:],
                            op=mybir.AluOpType.add)
                        if ti % 4 == 3 or ti == nbt - 1:
                            tb = t - ti % 4
                            nc.scalar.dma_start(
                                h_local[128 * tb:128 * (t + 1), :]
                                .rearrange("(a p) f -> p a f", p=128),
                                hb[:, :ti % 4 + 1, :])
                    r0 = BAND_START[k]
                    cc = nc.gpsimd.collective_compute(
                        "AllGather", mybir.AluOpType.bypass,
                        replica_groups=[list(range(N_CORES))],
                        ins=[h_local[r0:r0 + BANDS[k], :].opt()],
                        outs=[hgc[k][:].opt()])
                    ccs.append(cc)

            # expand each band's compact table into 256B rows, one engine
            # (hence one DMA queue) per band so the expands run in parallel
            exp_engines = [nc.sync, nc.scalar, nc.sync, nc.scalar]
            expands = []
            for k in range(N_BANDS):
                ex = exp_engines[k].dma_start(hg[k][:, 0:OUT_F], hgc[k][:])
                add_dep_helper(ex.ins, ccs[k].ins, reason="expand reads hgc")
                expands.append(ex)

            if h_dbg is not None:
                nc.sync.dma_start(h_dbg[:], h_local[:])

            # ---------------- phase 2: gather + scale + accumulate ----
            with tc.tile_pool(name="p2idx", bufs=3) as p2i, \
                 tc.tile_pool(name="p2g", bufs=3) as p2g, \
                 tc.tile_pool(name="p2b", bufs=3) as p2b, \
                 tc.tile_pool(name="p2w", bufs=2) as p2w, \
                 tc.tile_pool(name="p2psum", bufs=2, space="PSUM") as p2p, \
                 tc.tile_pool(name="p2out", bufs=3) as p2o:
                chunk_off = 0
                gq = 0                      # rotating SWDGE queue
                for gi, (t0, t1) in enumerate(groups):
                    meta = gather_meta[gi]
                    nch_g = sum(m[2] for m in meta)
                    ntg = t1 - t0
                    # idx + w for the whole group (contiguous columns)
                    icol0 = meta[0][0]
                    icols = sum(m[1] // 16 for m in meta)
                    idx_sb = p2i.tile([128, icols], mybir.dt.int16, tag="idx")
                    nc.sync.dma_start(idx_sb[:], idx_in[:, icol0:icol0 + icols])
                    w_sb = p2w.tile([128, nch_g], bf16, tag="w")
                    nc.sync.dma_start(w_sb[:], w_in[:, chunk_off:chunk_off + nch_g])

                    gtiles = []
                    for q in range(N_BANDS):
                        icol, nidx, nch = meta[q]
                        g_sb = p2g.tile([128, nch, HPAD], bf16, tag=f"g{q}",
                                        name=f"g{gi}_{q}")
                        qoff = icol - icol0
                        for b in range(0, nch, CHUNKS_PER_GATHER):
                            c1 = min(nch, b + CHUNKS_PER_GATHER)
                            ni = (c1 - b) * 128
                            gi_inst = nc.gpsimd.dma_gather(
                                out_ap=g_sb[:, b:c1, :],
                                in_ap=hg[q][:],
                                idxs_ap=idx_sb[:, qoff + b * 8:qoff + b * 8 + ni // 16],
                                num_idxs=ni,
                                num_idxs_reg=ni,
                                elem_size=HPAD,
                                single_packet=False,
                                queue_num=gq)
                            gq = (gq + 1) % N_SWDGE_QUEUES
                            add_dep_helper(gi_inst.ins, expands[q].ins,
                                           reason="gather reads hg band")
                        # scale to compact bf16: gb = g[:, :, :32] * w
                        wq0 = sum(m[2] for m in meta[:q])
                        gb = p2b.tile([128, nch, OUT_F], bf16, tag=f"gb{q}",
                                      name=f"gb{gi}_{q}")
                        nc.vector.tensor_tensor(
                            out=gb[:],
                            in0=g_sb[:, :, 0:OUT_F],
                            in1=w_sb[:, wq0:wq0 + nch].unsqueeze(2)
                                .to_broadcast([128, nch, OUT_F]),
                            op=mybir.AluOpType.mult)
                        gtiles.append(gb)

                    # accumulate per (band, tile) into PSUM
                    banks = [p2p.tile([128, ntg * OUT_F], f32,
                                      space="PSUM", tag=f"acc{q}",
                                      name=f"acc{gi}_{q}")
                             for q in range(N_BANDS)]
                    for q in range(N_BANDS):
                        gb = gtiles[q]
                        kk = 0
                        for t in range(t0, t1):
                            acc = banks[q][:, OUT_F * (t - t0):
                                           OUT_F * (t - t0 + 1)]
                            for k in range(int(K[q, t])):
                                nc.tensor.matmul(
                                    out=acc,
                                    lhsT=ident[:],
                                    rhs=gb[:, kk, :],
                                    start=(k == 0), stop=(k == int(K[q, t]) - 1))
                                kk += 1
                    for q in range(N_BANDS):
                        ob = p2o.tile([128, ntg * OUT_F], f32, tag="ob")
                        nc.scalar.copy(ob[:], banks[q][:])
                        nc.scalar.dma_start(
                            partial[q, :, t0:t1, :],
                            ob[:].rearrange("p (a f) -> p a f", f=OUT_F))
                    chunk_off += nch_g
    nc.compile()
    return nc


# ---- output combination ------------------------------------------------

def combine(partials, prep):
    """partials: list per core of [N_BANDS, 128, NTILES, OUT_F]
    (partition-major, permuted rows)."""
    out = np.empty((N_CORES * SHARD_RAW, OUT_F), np.float32)
    for c in range(N_CORES):
        p = np.asarray(partials[c])
        # [B, 128, T, F] -> [B, SHARD, F] with row r = 128*t + partition
        rows = p.transpose(0, 2, 1, 3).reshape(N_BANDS, SHARD, OUT_F)
        acc = rows[0][np.asarray(prep["rank"][c, 0], np.int64)]
        for q in range(1, N_BANDS):
            acc += rows[q][np.asarray(prep["rank"][c, q], np.int64)]
        out[c * SHARD_RAW:(c + 1) * SHARD_RAW] = acc[:SHARD_RAW]
    return out


# ---- entry point -------------------------------------------------------

_CACHE = {}


def kernel(x, weight, bias, edge_weight, edge_index):
    x = np.asarray(x, np.float32)
    weight = np.asarray(weight, np.float32)
    bias = np.asarray(bias, np.float32)
    edge_weight = np.asarray(edge_weight, np.float32)
    edge_index = np.asarray(edge_index, np.int32)

    prep = prepare(edge_index, edge_weight)
    # rank (inverse permutation) per (core, band) for combine()
    order = prep["order"]
    rank = np.empty_like(order)
    np.put_along_axis(
        rank, order, np.arange(SHARD, dtype=np.int32)[None, None, :], axis=2)
    prep["rank"] = rank

    key = (tuple(map(tuple, prep["K"])), tuple(prep["groups"]))
    if key not in _CACHE:
        _CACHE.clear()
        _CACHE[key] = build_nc(prep["K"], prep["groups"], prep["gather_meta"],
                               prep["total_chunks"], prep["total_slots"])
    nc = _CACHE[key]

    # host-side: pad, transpose, bf16-convert x; bf16 W
    xT = np.zeros((N_CORES, KTILES, 128, SHARD), ml_dtypes.bfloat16)
    for c in range(N_CORES):
        n0 = c * SHARD_RAW
        xt = _to_bf16(x[n0:n0 + SHARD_RAW].T)        # [512, 12500]
        xT[c, :, :, :SHARD_RAW] = xt.reshape(KTILES, 128, SHARD_RAW)
    wgt_b = _to_bf16(weight).reshape(KTILES, 128, OUT_F)

    in_maps = [{
        "xT": xT[c],
        "wgt": wgt_b,
        "bias_r": np.ascontiguousarray(
            np.broadcast_to(bias.reshape(1, OUT_F), (128, OUT_F))),
        "idx": prep["idx_in"][c],
        "wcol": prep["w_in"][c],
    } for c in range(N_CORES)]

    kw = {}
    if os.environ.get("KM_TRACE"):
        kw = dict(trace=True,
                  trace_cores=[int(c) for c in
                               os.environ.get("KM_TRACE_CORES", "0").split(",")],
                  tmpdir=os.environ.get("KM_TRACE_DIR") or None)
    res = bass_utils.run_bass_kernel_spmd(
        nc, in_maps, core_ids=list(range(N_CORES)), **kw)
    if os.environ.get("KM_TRACE"):
        global KM_LAST_RESULT
        KM_LAST_RESULT = res
    partials = [r["partial"] for r in res.results]
    return combine(partials, prep)


# revision 42
# speedup vs baseline: 1.0577x; 1.0577x over previous
"""GCN layer kernel for 8 Trainium2 NeuronCores.

Computes out = segment_sum(edge_weight * (x @ W + b)[src], dst) for a fixed
problem size: 100000 nodes, 1.6M edges, 512 -> 32 features.

Strategy (v5)
-------------
Phase 1 (per core): core c owns nodes [12500c, 12500(c+1)). The host supplies
x pre-transposed in bf16 (xT [512, SHARD]); the device computes
h = x @ W + b with xT k-chunks as the stationary operand (no on-device
transposes), writing h rows as bf16 in 256B rows (cols 32:128 garbage, never
read) into h_local.

The shard rows split into 4 BANDS (3200/3200/3200/2944 rows). Band k of all
8 cores forms gather table hg_k [8*n_k, 128] bf16 (<= 25600 rows: int16-safe
indices). Each band has its own AllGather that fires as soon as phase 1
finishes that band's rows, so collectives pipeline behind phase-1 compute and
the first gathers start ~60us in.

Phase 2 (per core): edges are routed to the core owning their dst. Per
(core, band), dst nodes are sorted by in-degree and packed into tiles of 128
"slots"; chunk k holds at most one edge per slot. dma_gather calls batch up
to 64 chunks (8192 indices) and rotate across 4 SWDGE queues so descriptor
generation overlaps DMA execution. Gathered chunks are scaled by edge weight
on DVE (bf16), then accumulated per tile with PSUM matmuls using a fixed
bf16 identity as the STATIONARY operand (messages move: 32 cols per chunk).
Each band accumulates into its own PSUM columns; the host sums the four band
partials after undoing the per-band permutations.

kernel() is self-contained: it takes the full inputs, shards them, compiles
the Bass program once (cached), runs it on cores 0-7 and reassembles the
full [100000, 32] output.
"""
import os

import numpy as np
import ml_dtypes

import concourse.bacc as bacc
import concourse.bass as bass
import concourse.tile as tile
from concourse import bass_utils, mybir
from concourse.masks import make_identity
from concourse.tile import add_dep_helper

# ---- problem constants -------------------------------------------------
N_NODES = 100000
N_EDGES = 1600000
IN_F = 512
OUT_F = 32

N_CORES = 8
SHARD_RAW = 12500          # real nodes per core
SHARD = 12544              # padded: 98 tiles of 128
NTILES = SHARD // 128      # 98
N_BANDS = 4
BANDS = [3200, 3200, 3200, 2944]        # rows per band (25/25/25/23 tiles)
BAND_START = [0, 3200, 6400, 9600]
BAND_TILES = [25, 25, 25, 23]
HPAD = 128                 # h row padded to 128 bf16 = 256B
KTILES = IN_F // 128       # 4 k-blocks in phase 1

CHUNKS_PER_GROUP = 160     # chunk budget per tile group (bounds SBUF)
CHUNKS_PER_GATHER = 64     # 8192 indices per dma_gather call
MAX_TILES_PER_GROUP = 16   # one PSUM bank holds 16 tiles x 32 f32 per band
N_SWDGE_QUEUES = 4


def _to_bf16(a):
    return np.ascontiguousarray(np.asarray(a, np.float32).astype(
        ml_dtypes.bfloat16))


# ---- host prep ---------------------------------------------------------

def prepare(edge_index, edge_weight):
    dst = np.asarray(edge_index[0], dtype=np.int64)
    src = np.asarray(edge_index[1], dtype=np.int64)
    w = np.asarray(edge_weight, dtype=np.float32)

    core_s = src // SHARD_RAW
    r_loc = src % SHARD_RAW
    band = ((r_loc >= BAND_START[1]).astype(np.int64)
            + (r_loc >= BAND_START[2]) + (r_loc >= BAND_START[3]))
    bands_arr = np.asarray(BANDS, np.int64)
    starts_arr = np.asarray(BAND_START, np.int64)
    loc = core_s * bands_arr[band] + (r_loc - starts_arr[band])

    core = dst // SHARD_RAW
    dst_local = dst % SHARD_RAW

    deg = np.zeros((N_CORES, N_BANDS, SHARD), np.int64)
    np.add.at(deg, (core, band, dst_local), 1)

    order = np.argsort(-deg, axis=2, kind="stable").astype(np.int32)
    rank = np.empty_like(order)
    np.put_along_axis(
        rank, order, np.arange(SHARD, dtype=np.int32)[None, None, :], axis=2)

    deg_sorted = np.take_along_axis(deg, order.astype(np.int64), axis=2)
    tile_max = deg_sorted.reshape(N_CORES, N_BANDS, NTILES, 128)[:, :, :, 0]
    K = np.maximum(tile_max.max(axis=0), 1).astype(np.int64)   # [B, T]

    # tile groups balanced by total chunk count across bands
    per_tile_chunks = K.sum(axis=0)                            # [T]
    groups = []                                                # (t0, t1)
    t0 = 0
    while t0 < NTILES:
        t1 = t0
        n = 0
        while (t1 < NTILES and t1 - t0 < MAX_TILES_PER_GROUP
               and (n == 0 or n + per_tile_chunks[t1] <= CHUNKS_PER_GROUP)):
            n += int(per_tile_chunks[t1])
            t1 += 1
        groups.append((t0, t1))
        t0 = t1

    # chunk order: (group, band, tile, k)
    chunk_base = np.zeros((N_BANDS, NTILES), np.int64)
    nxt = 0
    for (t0, t1) in groups:
        for q in range(N_BANDS):
            for t in range(t0, t1):
                chunk_base[q, t] = nxt
                nxt += int(K[q, t])
    total_chunks = nxt
    total_slots = total_chunks * 128

    # per-edge slot position
    e_rank = rank[core, band, dst_local].astype(np.int64)
    e_tile = e_rank // 128
    e_slot = e_rank % 128
    key = (core * N_BANDS + band) * SHARD + dst_local
    sort_idx = np.argsort(key, kind="stable")
    key_sorted = key[sort_idx]
    first = np.ones(len(key_sorted), bool)
    first[1:] = key_sorted[1:] != key_sorted[:-1]
    run_start = np.maximum.accumulate(
        np.where(first, np.arange(len(key_sorted)), 0))
    k_sorted = np.arange(len(key_sorted)) - run_start
    e_k = np.empty(len(key), np.int64)
    e_k[sort_idx] = k_sorted

    e_pos = (chunk_base[band, e_tile] + e_k) * 128 + e_slot

    idx_flat = np.zeros((N_CORES, total_slots), np.int16)
    w_flat = np.zeros((N_CORES, total_slots), np.float32)
    idx_flat[core, e_pos] = loc.astype(np.int16)
    w_flat[core, e_pos] = w

    # gather instruction meta: per (group, band): (idx col offset, nidx, nch)
    gather_meta = []
    icol = 0
    for (t0, t1) in groups:
        per_q = []
        for q in range(N_BANDS):
            nch = int(K[q, t0:t1].sum())
            nidx = nch * 128
            per_q.append((icol, nidx, nch))
            icol += nidx // 16
        gather_meta.append(per_q)

    # idx wrapped by 16 per (group, band) block, concatenated along cols.
    # slot layout is already (group, band, tile, k, slot) contiguous.
    idx_wrapped = (idx_flat.reshape(N_CORES, total_slots // 16, 16)
                   .transpose(0, 2, 1))                        # [C, 16, S]
    idx_in = np.ascontiguousarray(
        np.tile(idx_wrapped, (1, 8, 1)))                       # [C, 128, S]

    # per-chunk weights, broadcast layout [C, 128 slots, chunks], bf16
    w_in = np.ascontiguousarray(
        _to_bf16(w_flat).reshape(N_CORES, total_chunks, 128).transpose(0, 2, 1))

    return dict(K=K, groups=groups, chunk_base=chunk_base,
                total_chunks=total_chunks, total_slots=total_slots,
                gather_meta=gather_meta, idx_in=idx_in, w_in=w_in,
                order=order)


# ---- device program ----------------------------------------------------

def build_nc(K, groups, gather_meta, total_chunks, total_slots):
    nc = bacc.Bacc("TRN2", target_bir_lowering=False, debug=False,
                   num_devices=N_CORES, num_swdge_queues=N_SWDGE_QUEUES)
    f32 = mybir.dt.float32
    bf16 = mybir.dt.bfloat16
    # host supplies xT bf16: [KTILES, 128, SHARD]
    x_in = nc.dram_tensor("xT", [KTILES, 128, SHARD], bf16,
                          kind="ExternalInput")
    wgt_in = nc.dram_tensor("wgt", [KTILES, 128, OUT_F], bf16,
                            kind="ExternalInput")
    bias_in = nc.dram_tensor("bias_r", [128, OUT_F], f32, kind="ExternalInput")
    idx_in = nc.dram_tensor("idx", [128, total_slots // 16], mybir.dt.int16,
                            kind="ExternalInput")
    w_in = nc.dram_tensor("wcol", [128, total_chunks], bf16,
                          kind="ExternalInput")
    # per-band partial outputs, partition-major: [band, p, tile, f];
    # node 128*t + p of band q lives at partial[q, p, t, :]
    partial = nc.dram_tensor("partial", [N_BANDS, 128, NTILES, OUT_F], f32,
                             kind="ExternalOutput")
    h_dbg = None
    if os.environ.get("KM_DEBUG_H"):
        h_dbg = nc.dram_tensor("h_dbg", [SHARD, OUT_F], bf16,
                               kind="ExternalOutput")

    with tile.TileContext(nc) as tc:
        with tc.tile_pool(name="dram", bufs=1, space="DRAM") as dram, \
             tc.tile_pool(name="const", bufs=1) as const:
            ident = const.tile([128, 128], bf16)
            make_identity(nc, ident[:])
            wgt = const.tile([128, KTILES, OUT_F], bf16)
            for j in range(KTILES):
                nc.sync.dma_start(wgt[:, j, :], wgt_in[j])
            bias_sb = const.tile([128, OUT_F], f32)
            nc.sync.dma_start(bias_sb[:], bias_in[:])

            h_local = dram.tile([SHARD, OUT_F], bf16)
            hgc = [dram.tile([N_CORES * BANDS[k], OUT_F], bf16,
                             addr_space="Shared", name=f"hgc{k}")
                   for k in range(N_BANDS)]
            hg = [dram.tile([N_CORES * BANDS[k], HPAD], bf16,
                            name=f"hg{k}")
                  for k in range(N_BANDS)]

            # ------ phase 1 (banded) + pipelined per-band allgather ------
            ccs = []
            with tc.tile_pool(name="p1x", bufs=2) as p1x, \
                 tc.tile_pool(name="p1psum", bufs=4, space="PSUM") as p1p, \
                 tc.tile_pool(name="p1out", bufs=2) as p1o:
                for k in range(N_BANDS):
                    bt0 = BAND_START[k] // 128
                    nbt = BAND_TILES[k]
                    xt = p1x.tile([128, KTILES, BANDS[0]], bf16, tag="xt")
                    for j in range(KTILES):
                        nc.sync.dma_start(
                            xt[:, j, 0:BANDS[k]],
                            x_in[j, :,
                                 BAND_START[k]:BAND_START[k] + BANDS[k]])
                    hb = None
                    for ti in range(nbt):
                        t = bt0 + ti
                        h_ps = p1p.tile([128, OUT_F], f32, space="PSUM",
                                        tag="h_ps")
                        for j in range(KTILES):
                            nc.tensor.matmul(
                                out=h_ps[:],
                                lhsT=xt[:, j, 128 * ti:128 * (ti + 1)],
                                rhs=wgt[:, j, :],
                                start=(j == 0), stop=(j == KTILES - 1))
                        if ti % 4 == 0:
                            hb = p1o.tile([128, 4, OUT_F], bf16, tag="hb")
                        nc.vector.tensor_tensor(
                            out=hb[:, ti % 4, :], in0=h_ps[:],
                            in1=bias_sb[:],
                            op=mybir.AluOpType.add)
                        if ti % 4 == 3 or ti == nbt - 1:
                            tb = t - ti % 4
                            nc.scalar.dma_start(
                                h_local[128 * tb:128 * (t + 1), :]
                                .rearrange("(a p) f -> p a f", p=128),
                                hb[:, :ti % 4 + 1, :])
                    r0 = BAND_START[k]
                    cc = nc.gpsimd.collective_compute(
                        "AllGather", mybir.AluOpType.bypass,
                        replica_groups=[list(range(N_CORES))],
                        ins=[h_local[r0:r0 + BANDS[k], :].opt()],
                        outs=[hgc[k][:].opt()])
                    ccs.append(cc)

            # expand each band's compact table into 256B rows, one engine
            # (hence one DMA queue) per band so the expands run in parallel
            exp_engines = [nc.sync, nc.scalar, nc.sync, nc.scalar]
            expands = []
            for k in range(N_BANDS):
                ex = exp_engines[k].dma_start(hg[k][:, 0:OUT_F], hgc[k][:])
                add_dep_helper(ex.ins, ccs[k].ins, reason="expand reads hgc")
                expands.append(ex)

            if h_dbg is not None:
                nc.sync.dma_start(h_dbg[:], h_local[:])

            # ---------------- phase 2: gather + scale + accumulate ----
            with tc.tile_pool(name="p2idx", bufs=3) as p2i, \
                 tc.tile_pool(name="p2g", bufs=3) as p2g, \
                 tc.tile_pool(name="p2b", bufs=3) as p2b, \
                 tc.tile_pool(name="p2w", bufs=2) as p2w, \
                 tc.tile_pool(name="p2psum", bufs=2, space="PSUM") as p2p, \
                 tc.tile_pool(name="p2out", bufs=3) as p2o:
                chunk_off = 0
                gq = 0                      # rotating SWDGE queue
                for gi, (t0, t1) in enumerate(groups):
                    meta = gather_meta[gi]
                    nch_g = sum(m[2] for m in meta)
                    ntg = t1 - t0
                    # idx + w for the whole group (contiguous columns)
                    icol0 = meta[0][0]
                    icols = sum(m[1] // 16 for m in meta)
                    idx_sb = p2i.tile([128, icols], mybir.dt.int16, tag="idx")
                    nc.sync.dma_start(idx_sb[:], idx_in[:, icol0:icol0 + icols])
                    w_sb = p2w.tile([128, nch_g], bf16, tag="w")
                    nc.sync.dma_start(w_sb[:], w_in[:, chunk_off:chunk_off + nch_g])

                    gtiles = []
                    for q in range(N_BANDS):
                        icol, nidx, nch = meta[q]
                        g_sb = p2g.tile([128, nch, HPAD], bf16, tag=f"g{q}",
                                        name=f"g{gi}_{q}")
                        qoff = icol - icol0
                        for b in range(0, nch, CHUNKS_PER_GATHER):
                            c1 = min(nch, b + CHUNKS_PER_GATHER)
                            ni = (c1 - b) * 128
                            gi_inst = nc.gpsimd.dma_gather(
                                out_ap=g_sb[:, b:c1, :],
                                in_ap=hg[q][:],
                                idxs_ap=idx_sb[:, qoff + b * 8:qoff + b * 8 + ni // 16],
                                num_idxs=ni,
                                num_idxs_reg=ni,
                                elem_size=HPAD,
                                single_packet=False,
                                queue_num=gq)
                            gq = (gq + 1) % N_SWDGE_QUEUES
                            add_dep_helper(gi_inst.ins, expands[q].ins,
                                           reason="gather reads hg band")
                        # scale to compact bf16: gb = g[:, :, :32] * w
                        wq0 = sum(m[2] for m in meta[:q])
                        gb = p2b.tile([128, nch, OUT_F], bf16, tag=f"gb{q}",
                                      name=f"gb{gi}_{q}")
                        nc.vector.tensor_tensor(
                            out=gb[:],
                            in0=g_sb[:, :, 0:OUT_F],
                            in1=w_sb[:, wq0:wq0 + nch].unsqueeze(2)
                                .to_broadcast([128, nch, OUT_F]),
                            op=mybir.AluOpType.mult)
                        gtiles.append(gb)

                    # accumulate per (band, tile) into PSUM
                    banks = [p2p.tile([128, ntg * OUT_F], f32,
                                      space="PSUM", tag=f"acc{q}",
                                      name=f"acc{gi}_{q}")
                             for q in range(N_BANDS)]
                    for q in range(N_BANDS):
                        gb = gtiles[q]
                        kk = 0
                        for t in range(t0, t1):
                            acc = banks[q][:, OUT_F * (t - t0):
                                           OUT_F * (t - t0 + 1)]
                            for k in range(int(K[q, t])):
                                nc.tensor.matmul(
                                    out=acc,
                                    lhsT=ident[:],
                                    rhs=gb[:, kk, :],
                                    start=(k == 0), stop=(k == int(K[q, t]) - 1))
                                kk += 1
                    for q in range(N_BANDS):
                        ob = p2o.tile([128, ntg * OUT_F], f32, tag="ob")
                        nc.scalar.copy(ob[:], banks[q][:])
                        nc.scalar.dma_start(
                            partial[q, :, t0:t1, :],
                            ob[:].rearrange("p (a f) -> p a f", f=OUT_F))
                    chunk_off += nch_g
    nc.compile()
    return nc


# ---- output combination ------------------------------------------------

def combine(partials, prep):
    """partials: list per core of [N_BANDS, 128, NTILES, OUT_F]
    (partition-major, permuted rows)."""
    out = np.empty((N_CORES * SHARD_RAW, OUT_F), np.float32)
    for c in range(N_CORES):
        p = np.asarray(partials[c])
        # [B, 128, T, F] -> [B, SHARD, F] with row r = 128*t + partition
        rows = p.transpose(0, 2, 1, 3).reshape(N_BANDS, SHARD, OUT_F)
        acc = rows[0][np.asarray(prep["rank"][c, 0], np.int64)]
        for q in range(1, N_BANDS):
            acc += rows[q][np.asarray(prep["rank"][c, q], np.int64)]
        out[c * SHARD_RAW:(c + 1) * SHARD_RAW] = acc[:SHARD_RAW]
    return out


# ---- entry point -------------------------------------------------------

_CACHE = {}


def kernel(x, weight, bias, edge_weight, edge_index):
    x = np.asarray(x, np.float32)
    weight = np.asarray(weight, np.float32)
    bias = np.asarray(bias, np.float32)
    edge_weight = np.asarray(edge_weight, np.float32)
    edge_index = np.asarray(edge_index, np.int32)

    prep = prepare(edge_index, edge_weight)
    # rank (inverse permutation) per (core, band) for combine()
    order = prep["order"]
    rank = np.empty_like(order)
    np.put_along_axis(
        rank, order, np.arange(SHARD, dtype=np.int32)[None, None, :], axis=2)
    prep["rank"] = rank

    key = (tuple(map(tuple, prep["K"])), tuple(prep["groups"]))
    if key not in _CACHE:
        _CACHE.clear()
        _CACHE[key] = build_nc(prep["K"], prep["groups"], prep["gather_meta"],
                               prep["total_chunks"], prep["total_slots"])
    nc = _CACHE[key]

    # host-side: pad, transpose, bf16-convert x; bf16 W
    xT = np.zeros((N_CORES, KTILES, 128, SHARD), ml_dtypes.bfloat16)
    for c in range(N_CORES):
        n0 = c * SHARD_RAW
        xt = _to_bf16(x[n0:n0 + SHARD_RAW].T)        # [512, 12500]
        xT[c, :, :, :SHARD_RAW] = xt.reshape(KTILES, 128, SHARD_RAW)
    wgt_b = _to_bf16(weight).reshape(KTILES, 128, OUT_F)

    in_maps = [{
        "xT": xT[c],
        "wgt": wgt_b,
        "bias_r": np.ascontiguousarray(
            np.broadcast_to(bias.reshape(1, OUT_F), (128, OUT_F))),
        "idx": prep["idx_in"][c],
        "wcol": prep["w_in"][c],
    } for c in range(N_CORES)]

    kw = {}
    if os.environ.get("KM_TRACE"):
        kw = dict(trace=True,
                  trace_cores=[int(c) for c in
                               os.environ.get("KM_TRACE_CORES", "0").split(",")],
                  tmpdir=os.environ.get("KM_TRACE_DIR") or None)
    res = bass_utils.run_bass_kernel_spmd(
        nc, in_maps, core_ids=list(range(N_CORES)), **kw)
    if os.environ.get("KM_TRACE"):
        global KM_LAST_RESULT
        KM_LAST_RESULT = res
    partials = [r["partial"] for r in res.results]
    return combine(partials, prep)
